# revision 17
# baseline (speedup 1.0000x reference)
"""BatchHardTriplet loss kernel for Trainium2 (8 NeuronCores, SPMD).

Strategy
--------
The loss is permutation-invariant over rows, so the host sorts rows by label.
After sorting, each 1024-row block (one core) has all of its positives inside a
contiguous <=2048-column "window" of the sorted order. The host additionally
permutes the *columns* of the gathered operand per-core so the window occupies
columns [0, 2048) — this makes the kernel structure identical on all 8 cores
(pure SPMD, no dynamic addressing).

Per core:
  sim block = embB(128x1024 block, as lhsT chunks).T @ embA (128x8192 permuted)
  neg metric = sim - 4*eq      (eq mask only nonzero inside the window)
  hardest_neg_sim = row-max over all 8192 cols  (window tiles masked)
  hardest_pos_sim = row-min over window cols of (sim - 4*eq)  (+4 undone later)
The -4*eq mask is applied on the TensorEngine by accumulating an extra matmul
(identity @ mask_fp8) into the same PSUM region — zero VectorEngine cost.
The device returns per-row min/max; the host (which knows the labels) applies
validity (rows whose class has >=2 members and >=1 negative) and the final
relu/mean. Diagonal (self) pairs are label-equal, so the -4 mask removes them
from the neg max; for the pos min the self term (1-4=-3) loses to any real
positive (sim<1 => sim-4<-3), and rows with no real positive are zeroed by the
host validity mask anyway.
"""

import os
import sys
import numpy as np

sys.path.insert(0, "/opt/trn_rl_repo")

B = 8192
D = 128
M = 8            # cores
R = B // M       # 1024 rows per core
MC = R // 128    # 8 chunks of 128 rows per core
WIN = 2048       # window columns (4 x 512 tiles)
NT = B // 512    # 16 column tiles
MARGIN = 0.3

_CACHE = {}


def _build_program():
    """Build (once) the Bass program shared by all 8 cores."""
    if "nc" in _CACHE:
        return _CACHE["nc"]

    import concourse.bass as bass
    import concourse.bacc as bacc
    import concourse.mybir as mybir
    from concourse import tile

    f32 = mybir.dt.float32
    bf16 = mybir.dt.bfloat16
    fp16 = mybir.dt.float16
    fp8 = mybir.dt.float8e4
    Copy = mybir.ActivationFunctionType.Copy

    nc = bacc.Bacc(None, target_bir_lowering=False)

    embA = nc.dram_tensor("embA", [D, B], bf16, kind="ExternalInput")
    embB = nc.dram_tensor("embB", [D, R], bf16, kind="ExternalInput")
    masks = nc.dram_tensor("masks", [MC, 128, WIN], fp8, kind="ExternalInput")
    iden = nc.dram_tensor("iden", [128, 128], fp8, kind="ExternalInput")
    mins = nc.dram_tensor("mins", [128, MC], f32, kind="ExternalOutput")
    maxs = nc.dram_tensor("maxs", [128, MC], f32, kind="ExternalOutput")

    NG = NT // 4  # 4 psum groups per chunk, each [128, 2048] (4 banks)

    with tile.TileContext(nc) as tc:
        with (
            tc.tile_pool(name="big", bufs=1) as big,
            tc.tile_pool(name="mk", bufs=2) as mk,
            tc.tile_pool(name="ps", bufs=2, space="PSUM") as ps,
            tc.tile_pool(name="cp", bufs=2) as cp,
            tc.tile_pool(name="st", bufs=1) as st,
        ):
            # DMA order: first-matmul operands land first
            Bt = big.tile([D, R], bf16)
            nc.sync.dma_start(Bt[:], embB[:])
            A = [big.tile([D, 2048], bf16, name=f"A{s}") for s in range(4)]
            nc.sync.dma_start(A[0][:], embA[:, 0:2048])
            Id = big.tile([128, 128], fp8)
            nc.sync.dma_start(Id[:], iden[:])
            Mk0 = mk.tile([128, WIN], fp8, tag="mask", name="mask0")
            nc.sync.dma_start(Mk0[:], masks[0])
            for s in range(1, 4):
                nc.sync.dma_start(A[s][:], embA[:, s * 2048:(s + 1) * 2048])

            min_t = st.tile([128, MC], f32)
            max_t = st.tile([128, MC], f32)
            dummy_sink = st.tile([128, 2], f32)

            for mc in range(MC):
                if mc == 0:
                    Mk = Mk0
                else:
                    Mk = mk.tile([128, WIN], fp8, tag="mask", name=f"mask{mc}")
                    nc.sync.dma_start(Mk[:], masks[mc])
                lhsT = Bt[:, mc * 128:(mc + 1) * 128]
                halfs = []
                for g in range(NG):
                    P = ps.tile([128, 2048], f32, tag="psum", name=f"P{mc}_{g}")
                    # redundant warm-up matmuls into the region the real
                    # t=0 matmul overwrites (start=True): keeps the PE
                    # activity monitor busy so the clock stays at 2.4 GHz
                    for _ in range(10 if (mc == 0 and g == 0) else 1):
                        nc.tensor.matmul(
                            P[:, 0:512], lhsT, A[g][:, 0:512],
                            start=True, stop=True, skip_group_check=True)
                    for t in range(4):
                        nc.tensor.matmul(
                            P[:, t * 512:(t + 1) * 512],
                            lhsT,
                            A[g][:, t * 512:(t + 1) * 512],
                            start=True,
                            stop=(g != 0),
                        )
                    if g == 0:
                        # window group: accumulate -4*eq mask via identity matmul
                        for t in range(4):
                            nc.tensor.matmul(
                                P[:, t * 512:(t + 1) * 512],
                                Id[:],
                                Mk[:, t * 512:(t + 1) * 512],
                                start=False,
                                stop=True,
                            )
                        # hardest-positive: fp32 min straight from PSUM
                        nc.vector.tensor_reduce(
                            min_t[:, mc:mc + 1], P[:],
                            axis=mybir.AxisListType.X, op=mybir.AluOpType.min,
                        )
                    # drain PSUM via ScalarE as fp16 (DVE stays free for maxes)
                    C = cp.tile([128, 2048], fp16, tag="cp", name=f"C{mc}_{g}",
                                bufs=8)
                    nc.scalar.activation(C[:], P[:], Copy)
                    halfs.append(C)
                # fp16 TT-max tree on DVE (2x packed mode), fold, one reduce
                t0 = cp.tile([128, 2048], fp16, tag="t0", name=f"t0_{mc}")
                t1 = cp.tile([128, 2048], fp16, tag="t1", name=f"t1_{mc}")
                u = cp.tile([128, 2048], fp16, tag="u", name=f"u_{mc}")
                v = cp.tile([128, 1024], fp16, tag="v", name=f"v_{mc}")
                nc.vector.tensor_tensor(
                    t0[:], halfs[0][:], halfs[1][:], op=mybir.AluOpType.max)
                nc.vector.tensor_tensor(
                    t1[:], halfs[2][:], halfs[3][:], op=mybir.AluOpType.max)
                nc.vector.tensor_tensor(
                    u[:], t0[:], t1[:], op=mybir.AluOpType.max)
                nc.vector.tensor_tensor(
                    v[:], u[:, 0:1024], u[:, 1024:2048], op=mybir.AluOpType.max)
                nc.vector.tensor_reduce(
                    max_t[:, mc:mc + 1], v[:],
                    axis=mybir.AxisListType.X, op=mybir.AluOpType.max,
                )
            nc.sync.dma_start(mins[:], min_t[:])
            nc.sync.dma_start(maxs[:], max_t[:])

    nc.compile()
    _CACHE["nc"] = nc
    return nc


def _prep_inputs(emb, labels):
    """Sort by label, build per-core permuted operands + fp8 masks."""
    import ml_dtypes

    emb = np.asarray(emb, dtype=np.float32)
    labels = np.asarray(labels)
    order = np.argsort(labels, kind="stable")
    labs = labels[order]
    embs = emb[order]
    embT = np.ascontiguousarray(embs.T)  # [D, B]

    starts = np.searchsorted(labs, labs, side="left")
    ends = np.searchsorted(labs, labs, side="right")
    counts = ends - starts
    valid = (counts >= 2) & (counts < B)

    iden = np.eye(128, dtype=ml_dtypes.float8_e4m3)

    in_maps = []
    for c in range(M):
        r0 = c * R
        s = int(starts[r0])
        e = int(ends[r0 + R - 1])
        assert e - s <= WIN, f"class window span {e - s} exceeds {WIN}"
        w0 = min(s, B - WIN)
        perm = np.concatenate(
            [np.arange(w0, w0 + WIN), np.arange(0, w0), np.arange(w0 + WIN, B)]
        )
        embA = np.ascontiguousarray(embT[:, perm]).astype(ml_dtypes.bfloat16)
        embB = np.ascontiguousarray(embT[:, r0:r0 + R]).astype(ml_dtypes.bfloat16)
        lab_rows = labs[r0:r0 + R].reshape(MC, 128)
        lab_win = labs[w0:w0 + WIN]
        eq = lab_rows[:, :, None] == lab_win[None, None, :]
        masks = np.where(eq, np.float32(-4.0), np.float32(0.0)).astype(
            ml_dtypes.float8_e4m3
        )
        in_maps.append(
            {"embA": embA, "embB": embB, "masks": masks, "iden": iden}
        )
    return in_maps, valid


def _postprocess(results, valid):
    minv = np.zeros(B, dtype=np.float32)
    maxv = np.zeros(B, dtype=np.float32)
    for c, res in enumerate(results):
        # mins [128, MC, 2] / maxs [128, MC]: partition p, chunk mc -> sorted row
        mn = res["mins"]
        mx = res["maxs"]
        for mc in range(MC):
            rows = slice(c * R + mc * 128, c * R + mc * 128 + 128)
            minv[rows] = mn[:, mc]
            maxv[rows] = mx[:, mc]
    hp = 1.0 - (minv + 4.0)   # hardest positive distance
    hn = 1.0 - maxv           # hardest negative distance
    per_row = np.maximum(0.0, hp - hn + MARGIN)
    cnt = int(valid.sum())
    if cnt == 0:
        return np.float32(0.0)
    return np.float32(np.sum(per_row[valid]) / cnt)


def run_device(in_maps, trace=False):
    from concourse.bass_utils import run_bass_kernel_spmd

    nc = _build_program()
    return run_bass_kernel_spmd(nc, in_maps, list(range(M)), trace=trace)


def kernel(emb, labels):
    in_maps, valid = _prep_inputs(emb, labels)
    out = run_device(in_maps, trace=False)
    return _postprocess(out.results, valid)


if __name__ == "__main__":
    # smoke test with random data
    rng = np.random.default_rng(0)
    emb = rng.standard_normal((B, D)).astype(np.float32)
    emb /= np.linalg.norm(emb, axis=1, keepdims=True) + 1e-12
    labels = rng.integers(0, 512, B).astype(np.int32)
    print(kernel(emb, labels))


# revision 18
# speedup vs baseline: 1.1554x; 1.1554x over previous
"""BatchHardTriplet loss kernel for Trainium2 (8 NeuronCores, SPMD).

Strategy
--------
The loss is permutation-invariant over rows, so the host sorts rows by label.
After sorting, each 1024-row block (one core) has all of its positives inside a
contiguous <=2048-column "window" of the sorted order. The host additionally
permutes the *columns* of the gathered operand per-core so the window occupies
columns [0, 2048) — this makes the kernel structure identical on all 8 cores
(pure SPMD, no dynamic addressing).

Per core:
  sim block = embB(128x1024 block, as lhsT chunks).T @ embA (128x8192 permuted)
  neg metric = sim - 4*eq      (eq mask only nonzero inside the window)
  hardest_neg_sim = row-max over all 8192 cols  (window tiles masked)
  hardest_pos_sim = row-min over window cols of (sim - 4*eq)  (+4 undone later)
The -4*eq mask is applied on the TensorEngine by accumulating an extra matmul
(identity @ mask_fp8) into the same PSUM region — zero VectorEngine cost.
The device returns per-row min/max; the host (which knows the labels) applies
validity (rows whose class has >=2 members and >=1 negative) and the final
relu/mean. Diagonal (self) pairs are label-equal, so the -4 mask removes them
from the neg max; for the pos min the self term (1-4=-3) loses to any real
positive (sim<1 => sim-4<-3), and rows with no real positive are zeroed by the
host validity mask anyway.
"""

import os
import sys
import numpy as np

sys.path.insert(0, "/opt/trn_rl_repo")

B = 8192
D = 128
M = 8            # cores
R = B // M       # 1024 rows per core
MC = R // 128    # 8 chunks of 128 rows per core
WIN = 2048       # window columns (4 x 512 tiles)
NT = B // 512    # 16 column tiles
MARGIN = 0.3

_CACHE = {}


def _build_program():
    """Build (once) the Bass program shared by all 8 cores."""
    if "nc" in _CACHE:
        return _CACHE["nc"]

    import concourse.bass as bass
    import concourse.bacc as bacc
    import concourse.mybir as mybir
    from concourse import tile

    f32 = mybir.dt.float32
    bf16 = mybir.dt.bfloat16
    fp16 = mybir.dt.float16
    fp8 = mybir.dt.float8e4
    Copy = mybir.ActivationFunctionType.Copy

    nc = bacc.Bacc(None, target_bir_lowering=False)

    embA = nc.dram_tensor("embA", [D, B], bf16, kind="ExternalInput")
    embB = nc.dram_tensor("embB", [D, R], bf16, kind="ExternalInput")
    masks = nc.dram_tensor("masks", [MC, 128, WIN], fp8, kind="ExternalInput")
    iden = nc.dram_tensor("iden", [128, 128], fp8, kind="ExternalInput")
    mins = nc.dram_tensor("mins", [128, MC, 2], f32, kind="ExternalOutput")
    maxs = nc.dram_tensor("maxs", [128, MC], f32, kind="ExternalOutput")

    NG = NT // 2  # 8 psum groups per chunk, each [128, 1024] (2 banks)

    with tile.TileContext(nc) as tc:
        with (
            tc.tile_pool(name="big", bufs=1) as big,
            tc.tile_pool(name="mk", bufs=2) as mk,
            tc.tile_pool(name="ps", bufs=3, space="PSUM") as ps,
            tc.tile_pool(name="scr", bufs=1, space="PSUM") as scr,
            tc.tile_pool(name="cp", bufs=2) as cp,
            tc.tile_pool(name="st", bufs=1) as st,
        ):
            # DMA order: first-matmul operands land first
            Bt = big.tile([D, R], bf16)
            nc.sync.dma_start(Bt[:], embB[:])
            A = [big.tile([D, 2048], bf16, name=f"A{s}") for s in range(4)]
            nc.sync.dma_start(A[0][:], embA[:, 0:2048])
            Id = big.tile([128, 128], fp8)
            nc.sync.dma_start(Id[:], iden[:])
            Mk0 = mk.tile([128, WIN], fp8, tag="mask", name="mask0")
            nc.sync.dma_start(Mk0[:], masks[0])
            for s in range(1, 4):
                nc.sync.dma_start(A[s][:], embA[:, s * 2048:(s + 1) * 2048])

            min_t = st.tile([128, MC, 2], f32)
            max_a = st.tile([128, MC], f32)
            max_b = st.tile([128, MC], f32)
            max_t = st.tile([128, MC], f32)
            dummy_sink = st.tile([128, 2], f32)

            # scratch-bank matmuls keep the PE activity monitor busy so the
            # clock stays at 2.4 GHz despite drain-paced gaps
            S = scr.tile([128, 512], f32)

            def dummies(n):
                for _ in range(n):
                    nc.tensor.matmul(S[:], Bt[:, 0:128], A[0][:, 0:512],
                                     start=True, stop=True,
                                     skip_group_check=True)

            dummies(16)

            for mc in range(MC):
                if mc == 0:
                    Mk = Mk0
                else:
                    Mk = mk.tile([128, WIN], fp8, tag="mask", name=f"mask{mc}")
                    nc.sync.dma_start(Mk[:], masks[mc])
                lhsT = Bt[:, mc * 128:(mc + 1) * 128]
                halfs = []
                for g in range(NG):
                    P = ps.tile([128, 1024], f32, tag="psum", name=f"P{mc}_{g}")
                    for t in range(2):
                        nc.tensor.matmul(
                            P[:, t * 512:(t + 1) * 512],
                            lhsT,
                            A[g // 2][:, (g % 2) * 1024 + t * 512:
                                      (g % 2) * 1024 + (t + 1) * 512],
                            start=True,
                            stop=(g >= 2),
                        )
                    if g < 2:
                        # window group: accumulate -4*eq mask via identity matmul
                        for t in range(2):
                            nc.tensor.matmul(
                                P[:, t * 512:(t + 1) * 512],
                                Id[:],
                                Mk[:, g * 1024 + t * 512:
                                   g * 1024 + (t + 1) * 512],
                                start=False,
                                stop=True,
                            )
                        # hardest-positive: fp32 min straight from PSUM
                        nc.vector.tensor_reduce(
                            min_t[:, mc, g:g + 1], P[:],
                            axis=mybir.AxisListType.X, op=mybir.AluOpType.min,
                        )
                    if g == 7:
                        # last group: DVE reduces it directly (ACT offload)
                        nc.vector.tensor_reduce(
                            max_a[:, mc:mc + 1], P[:],
                            axis=mybir.AxisListType.X, op=mybir.AluOpType.max,
                        )
                    else:
                        # drain PSUM via ScalarE as fp16
                        C = cp.tile([128, 1024], fp16, tag="cp",
                                    name=f"C{mc}_{g}", bufs=14)
                        nc.scalar.activation(C[:], P[:], Copy)
                        halfs.append(C)
                    dummies(1)
                # fp16 TT-max tree on DVE (2x packed mode) over 7 halfs
                lvl = halfs
                li = 0
                while len(lvl) > 1:
                    nxt = []
                    for j in range(0, len(lvl) - 1, 2):
                        o = cp.tile([128, 1024], fp16, tag=f"t{li}_{j}",
                                    name=f"t{mc}_{li}_{j}", bufs=2)
                        nc.vector.tensor_tensor(
                            o[:], lvl[j][:], lvl[j + 1][:],
                            op=mybir.AluOpType.max)
                        nxt.append(o)
                    if len(lvl) % 2:
                        nxt.append(lvl[-1])
                    lvl = nxt
                    li += 1
                nc.vector.tensor_reduce(
                    max_b[:, mc:mc + 1], lvl[0][:],
                    axis=mybir.AxisListType.X, op=mybir.AluOpType.max,
                )
            nc.vector.tensor_tensor(
                max_t[:], max_a[:], max_b[:], op=mybir.AluOpType.max)
            nc.sync.dma_start(mins[:], min_t[:])
            nc.sync.dma_start(maxs[:], max_t[:])
            nc.vector.tensor_reduce(
                dummy_sink[:, 1:2], S[:], axis=mybir.AxisListType.X,
                op=mybir.AluOpType.max,
            )

    nc.compile()
    _CACHE["nc"] = nc
    return nc


def _prep_inputs(emb, labels):
    """Sort by label, build per-core permuted operands + fp8 masks."""
    import ml_dtypes

    emb = np.asarray(emb, dtype=np.float32)
    labels = np.asarray(labels)
    order = np.argsort(labels, kind="stable")
    labs = labels[order]
    embs = emb[order]
    embT = np.ascontiguousarray(embs.T)  # [D, B]

    starts = np.searchsorted(labs, labs, side="left")
    ends = np.searchsorted(labs, labs, side="right")
    counts = ends - starts
    valid = (counts >= 2) & (counts < B)

    iden = np.eye(128, dtype=ml_dtypes.float8_e4m3)

    in_maps = []
    for c in range(M):
        r0 = c * R
        s = int(starts[r0])
        e = int(ends[r0 + R - 1])
        assert e - s <= WIN, f"class window span {e - s} exceeds {WIN}"
        w0 = min(s, B - WIN)
        perm = np.concatenate(
            [np.arange(w0, w0 + WIN), np.arange(0, w0), np.arange(w0 + WIN, B)]
        )
        embA = np.ascontiguousarray(embT[:, perm]).astype(ml_dtypes.bfloat16)
        embB = np.ascontiguousarray(embT[:, r0:r0 + R]).astype(ml_dtypes.bfloat16)
        lab_rows = labs[r0:r0 + R].reshape(MC, 128)
        lab_win = labs[w0:w0 + WIN]
        eq = lab_rows[:, :, None] == lab_win[None, None, :]
        masks = np.where(eq, np.float32(-4.0), np.float32(0.0)).astype(
            ml_dtypes.float8_e4m3
        )
        in_maps.append(
            {"embA": embA, "embB": embB, "masks": masks, "iden": iden}
        )
    return in_maps, valid


def _postprocess(results, valid):
    minv = np.zeros(B, dtype=np.float32)
    maxv = np.zeros(B, dtype=np.float32)
    for c, res in enumerate(results):
        # mins [128, MC, 2] / maxs [128, MC]: partition p, chunk mc -> sorted row
        mn = res["mins"].min(axis=2)
        mx = res["maxs"]
        for mc in range(MC):
            rows = slice(c * R + mc * 128, c * R + mc * 128 + 128)
            minv[rows] = mn[:, mc]
            maxv[rows] = mx[:, mc]
    hp = 1.0 - (minv + 4.0)   # hardest positive distance
    hn = 1.0 - maxv           # hardest negative distance
    per_row = np.maximum(0.0, hp - hn + MARGIN)
    cnt = int(valid.sum())
    if cnt == 0:
        return np.float32(0.0)
    return np.float32(np.sum(per_row[valid]) / cnt)


def run_device(in_maps, trace=False):
    from concourse.bass_utils import run_bass_kernel_spmd

    nc = _build_program()
    return run_bass_kernel_spmd(nc, in_maps, list(range(M)), trace=trace)


def kernel(emb, labels):
    in_maps, valid = _prep_inputs(emb, labels)
    out = run_device(in_maps, trace=False)
    return _postprocess(out.results, valid)


if __name__ == "__main__":
    # smoke test with random data
    rng = np.random.default_rng(0)
    emb = rng.standard_normal((B, D)).astype(np.float32)
    emb /= np.linalg.norm(emb, axis=1, keepdims=True) + 1e-12
    labels = rng.integers(0, 512, B).astype(np.int32)
    print(kernel(emb, labels))


# revision 20
# speedup vs baseline: 1.2477x; 1.0799x over previous
"""BatchHardTriplet loss kernel for Trainium2 (8 NeuronCores, SPMD).

Strategy
--------
The loss is permutation-invariant over rows, so the host sorts rows by label.
After sorting, each 1024-row block (one core) has all of its positives inside a
contiguous <=2048-column "window" of the sorted order. The host additionally
permutes the *columns* of the gathered operand per-core so the window occupies
columns [0, 2048) — this makes the kernel structure identical on all 8 cores
(pure SPMD, no dynamic addressing).

Per core:
  sim block = embB(128x1024 block, as lhsT chunks).T @ embA (128x8192 permuted)
  neg metric = sim - 4*eq      (eq mask only nonzero inside the window)
  hardest_neg_sim = row-max over all 8192 cols  (window tiles masked)
  hardest_pos_sim = row-min over window cols of (sim - 4*eq)  (+4 undone later)
The -4*eq mask is applied on the TensorEngine by accumulating an extra matmul
(identity @ mask_fp8) into the same PSUM region — zero VectorEngine cost.
The device returns per-row min/max; the host (which knows the labels) applies
validity (rows whose class has >=2 members and >=1 negative) and the final
relu/mean. Diagonal (self) pairs are label-equal, so the -4 mask removes them
from the neg max; for the pos min the self term (1-4=-3) loses to any real
positive (sim<1 => sim-4<-3), and rows with no real positive are zeroed by the
host validity mask anyway.
"""

import os
import sys
import numpy as np

sys.path.insert(0, "/opt/trn_rl_repo")

B = 8192
D = 128
M = 8            # cores
R = B // M       # 1024 rows per core
MC = R // 128    # 8 chunks of 128 rows per core
WIN = 2048       # window columns (4 x 512 tiles)
NT = B // 512    # 16 column tiles
MARGIN = 0.3

_CACHE = {}


def _build_program():
    """Build (once) the Bass program shared by all 8 cores."""
    if "nc" in _CACHE:
        return _CACHE["nc"]

    import concourse.bass as bass
    import concourse.bacc as bacc
    import concourse.mybir as mybir
    from concourse import tile

    f32 = mybir.dt.float32
    bf16 = mybir.dt.bfloat16
    fp16 = mybir.dt.float16
    fp8 = mybir.dt.float8e4
    Copy = mybir.ActivationFunctionType.Copy

    nc = bacc.Bacc(None, target_bir_lowering=False)

    embA = nc.dram_tensor("embA", [D, B], bf16, kind="ExternalInput")
    embB = nc.dram_tensor("embB", [D, R], bf16, kind="ExternalInput")
    masks = nc.dram_tensor("masks", [MC, 128, WIN], fp8, kind="ExternalInput")
    iden = nc.dram_tensor("iden", [128, 128], fp8, kind="ExternalInput")
    mins = nc.dram_tensor("mins", [128, MC, 2], f32, kind="ExternalOutput")
    maxs = nc.dram_tensor("maxs", [128, MC], f32, kind="ExternalOutput")

    NG = NT // 2  # 8 psum groups per chunk, each [128, 1024] (2 banks)

    with tile.TileContext(nc) as tc:
        with (
            tc.tile_pool(name="big", bufs=1) as big,
            tc.tile_pool(name="mk", bufs=2) as mk,
            tc.tile_pool(name="ps", bufs=3, space="PSUM") as ps,
            tc.tile_pool(name="scr", bufs=1, space="PSUM") as scr,
            tc.tile_pool(name="cp", bufs=2) as cp,
            tc.tile_pool(name="st", bufs=1) as st,
        ):
            # DMA order: first-matmul operands land first
            Bt = big.tile([D, R], bf16)
            nc.sync.dma_start(Bt[:], embB[:])
            A = [big.tile([D, 2048], bf16, name=f"A{s}") for s in range(4)]
            nc.sync.dma_start(A[0][:], embA[:, 0:2048])
            Id = big.tile([128, 128], fp8)
            nc.sync.dma_start(Id[:], iden[:])
            Mk0 = mk.tile([128, WIN], fp8, tag="mask", name="mask0")
            nc.sync.dma_start(Mk0[:], masks[0])
            for s in range(1, 4):
                nc.sync.dma_start(A[s][:], embA[:, s * 2048:(s + 1) * 2048])

            min_t = st.tile([128, MC, 2], f32)
            max_a = st.tile([128, MC], f32)
            max_b = st.tile([128, MC], f32)
            max_t = st.tile([128, MC], f32)
            dummy_sink = st.tile([128, 2], f32)

            # scratch-bank matmuls keep the PE activity monitor busy so the
            # clock stays at 2.4 GHz despite drain-paced gaps
            S = scr.tile([128, 512], f32)

            def dummies(n):
                for _ in range(n):
                    nc.tensor.matmul(S[:], Bt[:, 0:128], A[0][:, 0:512],
                                     start=True, stop=True,
                                     skip_group_check=True)

            dummies(6)

            for mc in range(MC):
                if mc == 0:
                    Mk = Mk0
                else:
                    Mk = mk.tile([128, WIN], fp8, tag="mask", name=f"mask{mc}")
                    nc.sync.dma_start(Mk[:], masks[mc])
                lhsT = Bt[:, mc * 128:(mc + 1) * 128]
                halfs = []
                for g in range(NG):
                    P = ps.tile([128, 1024], f32, tag="psum", name=f"P{mc}_{g}")
                    for t in range(2):
                        nc.tensor.matmul(
                            P[:, t * 512:(t + 1) * 512],
                            lhsT,
                            A[g // 2][:, (g % 2) * 1024 + t * 512:
                                      (g % 2) * 1024 + (t + 1) * 512],
                            start=True,
                            stop=(g >= 2),
                        )
                    if g < 2:
                        # window group: accumulate -4*eq mask via identity matmul
                        for t in range(2):
                            nc.tensor.matmul(
                                P[:, t * 512:(t + 1) * 512],
                                Id[:],
                                Mk[:, g * 1024 + t * 512:
                                   g * 1024 + (t + 1) * 512],
                                start=False,
                                stop=True,
                            )
                        # hardest-positive: fp32 min straight from PSUM.
                        # host guarantees all positives lie in window cols
                        # [0, 1536), so g1 only needs its first 512 cols
                        nc.vector.tensor_reduce(
                            min_t[:, mc, g:g + 1],
                            P[:] if g == 0 else P[:, 0:512],
                            axis=mybir.AxisListType.X, op=mybir.AluOpType.min,
                        )
                    if g == 7:
                        # last group: DVE reduces it directly (ACT offload)
                        nc.vector.tensor_reduce(
                            max_a[:, mc:mc + 1], P[:],
                            axis=mybir.AxisListType.X, op=mybir.AluOpType.max,
                        )
                    else:
                        # drain PSUM via ScalarE as fp16
                        C = cp.tile([128, 1024], fp16, tag="cp",
                                    name=f"C{mc}_{g}", bufs=14)
                        nc.scalar.activation(C[:], P[:], Copy)
                        halfs.append(C)
                    dummies(1)
                # fp16 TT-max tree on DVE (2x packed mode) over 7 halfs
                lvl = halfs
                li = 0
                while len(lvl) > 1:
                    nxt = []
                    for j in range(0, len(lvl) - 1, 2):
                        o = cp.tile([128, 1024], fp16, tag=f"t{li}_{j}",
                                    name=f"t{mc}_{li}_{j}", bufs=2)
                        nc.vector.tensor_tensor(
                            o[:], lvl[j][:], lvl[j + 1][:],
                            op=mybir.AluOpType.max)
                        nxt.append(o)
                    if len(lvl) % 2:
                        nxt.append(lvl[-1])
                    lvl = nxt
                    li += 1
                nc.vector.tensor_reduce(
                    max_b[:, mc:mc + 1], lvl[0][:],
                    axis=mybir.AxisListType.X, op=mybir.AluOpType.max,
                )
            nc.vector.tensor_tensor(
                max_t[:], max_a[:], max_b[:], op=mybir.AluOpType.max)
            nc.sync.dma_start(mins[:], min_t[:])
            nc.sync.dma_start(maxs[:], max_t[:])
            nc.vector.tensor_reduce(
                dummy_sink[:, 1:2], S[:], axis=mybir.AxisListType.X,
                op=mybir.AluOpType.max,
            )

    nc.compile()
    _CACHE["nc"] = nc
    return nc


def _prep_inputs(emb, labels):
    """Sort by label, build per-core permuted operands + fp8 masks."""
    import ml_dtypes

    emb = np.asarray(emb, dtype=np.float32)
    labels = np.asarray(labels)
    order = np.argsort(labels, kind="stable")
    labs = labels[order]
    embs = emb[order]
    embT = np.ascontiguousarray(embs.T)  # [D, B]

    starts = np.searchsorted(labs, labs, side="left")
    ends = np.searchsorted(labs, labs, side="right")
    counts = ends - starts
    valid = (counts >= 2) & (counts < B)

    iden = np.eye(128, dtype=ml_dtypes.float8_e4m3)

    in_maps = []
    for c in range(M):
        r0 = c * R
        s = int(starts[r0])
        e = int(ends[r0 + R - 1])
        assert e - s <= 1536, f"class window span {e - s} exceeds 1536"
        # rotate columns so the core's class span starts at window col 0:
        # all positives land in [0, span) with span <= 1536
        perm = (s + np.arange(B)) % B
        embA = np.ascontiguousarray(embT[:, perm]).astype(ml_dtypes.bfloat16)
        embB = np.ascontiguousarray(embT[:, r0:r0 + R]).astype(ml_dtypes.bfloat16)
        lab_rows = labs[r0:r0 + R].reshape(MC, 128)
        lab_win = labs[perm[:WIN]]
        eq = lab_rows[:, :, None] == lab_win[None, None, :]
        masks = np.where(eq, np.float32(-4.0), np.float32(0.0)).astype(
            ml_dtypes.float8_e4m3
        )
        in_maps.append(
            {"embA": embA, "embB": embB, "masks": masks, "iden": iden}
        )
    return in_maps, valid


def _postprocess(results, valid):
    minv = np.zeros(B, dtype=np.float32)
    maxv = np.zeros(B, dtype=np.float32)
    for c, res in enumerate(results):
        # mins [128, MC, 2] / maxs [128, MC]: partition p, chunk mc -> sorted row
        mn = res["mins"].min(axis=2)
        mx = res["maxs"]
        for mc in range(MC):
            rows = slice(c * R + mc * 128, c * R + mc * 128 + 128)
            minv[rows] = mn[:, mc]
            maxv[rows] = mx[:, mc]
    hp = 1.0 - (minv + 4.0)   # hardest positive distance
    hn = 1.0 - maxv           # hardest negative distance
    per_row = np.maximum(0.0, hp - hn + MARGIN)
    cnt = int(valid.sum())
    if cnt == 0:
        return np.float32(0.0)
    return np.float32(np.sum(per_row[valid]) / cnt)


def run_device(in_maps, trace=False):
    from concourse.bass_utils import run_bass_kernel_spmd

    nc = _build_program()
    return run_bass_kernel_spmd(nc, in_maps, list(range(M)), trace=trace)


def kernel(emb, labels):
    in_maps, valid = _prep_inputs(emb, labels)
    out = run_device(in_maps, trace=False)
    return _postprocess(out.results, valid)


if __name__ == "__main__":
    # smoke test with random data
    rng = np.random.default_rng(0)
    emb = rng.standard_normal((B, D)).astype(np.float32)
    emb /= np.linalg.norm(emb, axis=1, keepdims=True) + 1e-12
    labels = rng.integers(0, 512, B).astype(np.int32)
    print(kernel(emb, labels))
